# revision 1
# baseline (speedup 1.0000x reference)
"""BatchHardTripletLoss on 8 trn2 NeuronCores.

Strategy (anchor-row sharding, per sharding hint):
  - Host sorts rows by label; same-label columns become contiguous runs.
  - Each core owns 512 anchor rows and scans all 4096 columns.
  - PE builds psum_neg[i,j] = xi.xj - sqn_j/2  (= -H/2, H = sqn_j - 2 xi.xj)
    via fp32r matmuls: K=128 main + K=1 rank-1 accumulate for the sqn_j term.
  - hardest-neg: custom-DVE TENSOR_MASK_REDUCE over two 2048-wide psum
    groups with per-row wrapped masks that exclude the same-label run;
    accum = max over negatives of -H/2  =>  min over negatives of H.
    Fused mask+reduce: each matrix element is touched exactly once on DVE.
  - hardest-pos: the same-label run lives in a 512-wide window around the
    diagonal; a +H/2 psum over that window + a normal range mask, max.
  - Device returns per-(row-block) accums [128, 8]; host finishes the O(B)
    epilogue: hn2 = sqn_i - 2*accn, hp2 = sqn_i + 2*accp, sqrt, relu,
    valid-masking, and the final mean.
"""

import sys

if "/opt/trn_rl_repo" not in sys.path:
    sys.path.insert(0, "/opt/trn_rl_repo")

import numpy as np

B = 4096
D = 128
NCORES = 8
RPC = B // NCORES          # rows per core = 512
NRB = RPC // 128           # row blocks per core = 4
GRP = 2048                 # columns per neg psum group
NG = B // GRP              # 4 neg groups
POSW = 256                 # pos window width (max class run ~30 << 64)
MARGIN = 0.3

_cached = {}


def _build_program(repeat=1, hw_loop=0, variant="full"):
    import concourse.bacc as bacc
    import concourse.mybir as mybir
    from concourse import tile
    from concourse.dve_ops import TENSOR_MASK_REDUCE

    dt = mybir.dt
    f32 = dt.float32
    f32r = dt.float32r

    nc = bacc.Bacc(None, target_bir_lowering=False)

    bf16 = dt.bfloat16
    xt_d = nc.dram_tensor("xt", [D, B], f32r, kind="ExternalInput")
    sqn2_d = nc.dram_tensor("sqn2", [2, B], bf16, kind="ExternalInput")
    rowx_d = nc.dram_tensor("rowx", [D, RPC], f32r, kind="ExternalInput")
    negones_d = nc.dram_tensor("negones", [2, 128], bf16, kind="ExternalInput")
    ns_d = nc.dram_tensor("ns", [128, NG * NRB], f32, kind="ExternalInput")
    ne_d = nc.dram_tensor("ne", [128, NG * NRB], f32, kind="ExternalInput")
    ps_d = nc.dram_tensor("ps", [128, NRB], f32, kind="ExternalInput")
    pe_d = nc.dram_tensor("pe", [128, NRB], f32, kind="ExternalInput")
    acc_d = nc.dram_tensor("acc", [128, (NG + 1) * NRB], f32, kind="ExternalOutput")

    # pos window base per (global) row block: clip(128m - 192, 0, B - POSW).
    # Same formula on the host side; data-independent so the IR is static.
    def w0_of(mg):
        return min(max(128 * mg - 192, 0), B - POSW)

    with tile.TileContext(nc) as tc:
        with (
            tc.tile_pool(name="xtp", bufs=1) as xtp,
            tc.tile_pool(name="small", bufs=1) as small,
            tc.tile_pool(name="scr", bufs=4) as scrp,
            tc.tile_pool(name="scrps", bufs=4) as scrpp,
            tc.tile_pool(name="psum", bufs=2, space="PSUM") as psum,
        ):
            # ---- loads. Order matters: the first sqn matmuls need only
            # negones + sqnh; the first mains need rowx + xt chunk 0.
            negones = small.tile([2, 128], bf16, tag="negones")
            nc.sync.dma_start(negones[:], negones_d[:])
            sqn2 = small.tile([2, B], bf16, tag="sqn2")
            nc.scalar.dma_start(sqn2[:], sqn2_d[:])
            rowx = small.tile([D, RPC], f32r, tag="rowx")
            nc.sync.dma_start(rowx[:], rowx_d[:])
            xt = xtp.tile([D, B], f32r, tag="xt")
            for k in range(B // 512):
                eng = nc.sync if k % 2 == 0 else nc.scalar
                eng.dma_start(xt[:, 512 * k : 512 * (k + 1)],
                              xt_d[:, 512 * k : 512 * (k + 1)])
            ns = small.tile([128, NG * NRB], f32, tag="ns")
            ne = small.tile([128, NG * NRB], f32, tag="ne")
            ps = small.tile([128, NRB], f32, tag="ps")
            pe = small.tile([128, NRB], f32, tag="pe")
            for t, d in ((ns, ns_d), (ne, ne_d), (ps, ps_d), (pe, pe_d)):
                nc.gpsimd.dma_start(t[:], d[:])

            acc = small.tile([128, (NG + 1) * NRB], f32, tag="acc")
            nc.gpsimd.memset(acc[:], 0.0)

            # pos windows: absolute columns depend on the core id, so the
            # host pre-gathers this core's 4 windows, NEGATED, so the pos
            # psum (+H/2) can reuse the rowx stationary and negones:
            #   psum = negones @ (-sqn_w/2)  +  rowx.T @ (-x_w)  =  +H/2.
            nposx_d = nc.dram_tensor("nposx", [D, NRB * POSW], f32r,
                                     kind="ExternalInput")
            npossqn_d = nc.dram_tensor("npossqn", [2, NRB * POSW], bf16,
                                       kind="ExternalInput")
            nposx = small.tile([D, NRB * POSW], f32r, tag="nposx")
            npossqn = small.tile([2, NRB * POSW], bf16, tag="npossqn")
            nc.scalar.dma_start(nposx[:], nposx_d[:])
            nc.gpsimd.dma_start(npossqn[:], npossqn_d[:])

            def neg_body():
              for m in range(NRB * repeat):
                m = m % NRB
                stat = rowx[:, 128 * m : 128 * (m + 1)]

                # ---- neg groups: NG x GRP columns, fine-grained psum
                # rotation (bufs=4) keeps PE slot-waits short so HAM stays
                # warm. ----
                for g in range(NG):
                    big = psum.tile([128, GRP], f32, tag="BIG")
                    if variant != "mmmain":
                      for k in range(GRP // 512):
                        ck = g * (GRP // 512) + k
                        nc.tensor.matmul(
                            big[:, 512 * k : 512 * (k + 1)], negones[:],
                            sqn2[0:2, 512 * ck : 512 * (ck + 1)],
                            start=True, stop=(variant in ("dve", "mmsqn")),
                        )
                    if variant in ("full", "mm", "mmmain"):
                      for k in range(GRP // 512):
                        ck = g * (GRP // 512) + k
                        nc.tensor.matmul(
                            big[:, 512 * k : 512 * (k + 1)], stat,
                            xt[:, 512 * ck : 512 * (ck + 1)],
                            start=(variant == "mmmain"), stop=True,
                        )
                    if variant in ("mm", "mmmain", "mmsqn"):
                        continue
                    scr = scrp.tile([128, GRP], bf16, tag="scr")
                    col = NRB * g + m
                    nc.vector._custom_dve(
                        TENSOR_MASK_REDUCE,
                        out=scr[:],
                        in0=big[:],
                        in1=ne[:, col : col + 1],
                        s0=ns[:, col : col + 1],
                        s1=-3.0e38,
                        imm2=1.0,
                        accum_out=acc[:, col : col + 1],
                    )

            def pos_body():
              for m in range(NRB * repeat):
                m = m % NRB
                stat = rowx[:, 128 * m : 128 * (m + 1)]
                pos = psum.tile([128, POSW], f32, tag="BIG")
                if variant != "mmmain":
                    nc.tensor.matmul(pos[:], negones[:],
                                     npossqn[0:2, POSW * m : POSW * (m + 1)],
                                     start=True,
                                     stop=(variant in ("dve", "mmsqn")))
                if variant in ("full", "mm", "mmmain"):
                    nc.tensor.matmul(pos[:], stat,
                                     nposx[:, POSW * m : POSW * (m + 1)],
                                     start=(variant == "mmmain"), stop=True)
                if variant in ("mm", "mmmain", "mmsqn"):
                    continue
                scrps = scrpp.tile([128, POSW], bf16, tag="scrps")
                nc.vector._custom_dve(
                    TENSOR_MASK_REDUCE,
                    out=scrps[:],
                    in0=pos[:],
                    in1=pe[:, m : m + 1],
                    s0=ps[:, m : m + 1],
                    s1=-3.0e38,
                    imm2=1.0,
                    accum_out=acc[:, NG * NRB + m : NG * NRB + m + 1],
                )

            if hw_loop:
                with tc.For_i(0, hw_loop, 1):
                    neg_body()
                    pos_body()
            else:
                neg_body()
                pos_body()

            nc.sync.dma_start(acc_d[:], acc[:])

    nc.compile()
    return nc


def _prepare(embeddings, labels):
    E = np.asarray(embeddings, dtype=np.float32)
    L = np.asarray(labels).astype(np.int64)
    order = np.argsort(L, kind="stable")
    Es = E[order]
    Ls = L[order]

    sqn = (Es * Es).sum(axis=1, dtype=np.float32)
    xt = np.ascontiguousarray(Es.T)

    change = np.flatnonzero(np.diff(Ls)) + 1
    starts = np.concatenate([[0], change])
    ends = np.concatenate([change, [B]])
    run_id = np.zeros(B, dtype=np.int64)
    run_id[change] = 1
    run_id = np.cumsum(run_id)
    cs = starts[run_id].astype(np.int64)
    ce = ends[run_id].astype(np.int64)
    counts = ce - cs
    valid = ((counts >= 2) & (counts <= B - 1)).astype(np.float64)
    n_valid = valid.sum()

    def neg_args(g0):
        lcs = np.clip(cs - g0, 0, GRP).astype(np.float64)
        lce = np.clip(ce - g0, 0, GRP).astype(np.float64)
        ms = lce.copy()
        me = lcs.copy()
        empty = lcs == lce
        ms[empty] = GRP + 6.0
        me[empty] = GRP + 5.0
        return ms.astype(np.float32), me.astype(np.float32)

    negms, negme = zip(*(neg_args(GRP * g) for g in range(NG)))

    mglob = np.arange(B // 128)
    w0 = np.clip(128 * mglob - 64, 0, B - POSW)
    w0r = np.repeat(w0, 128)
    assert (cs >= w0r).all() and (ce <= w0r + POSW).all(), "pos window overflow"
    pstart = (cs - w0r).astype(np.float32)
    pend = (ce - w0r).astype(np.float32)

    return dict(
        order=order, sqn=sqn, xt=xt, valid=valid, n_valid=n_valid,
        negms=negms, negme=negme,
        pstart=pstart, pend=pend, w0=w0,
    )


def _core_cols(vec, c):
    # [B] -> [128, NRB] with [p, m] = vec[512c + 128m + p]
    return np.ascontiguousarray(
        vec[RPC * c : RPC * (c + 1)].reshape(NRB, 128).T
    )


def _hilo(x):
    import ml_dtypes
    hi = x.astype(ml_dtypes.bfloat16)
    lo = (x - hi.astype(np.float32)).astype(ml_dtypes.bfloat16)
    return np.stack([hi, lo])


def _in_maps(pre):
    import ml_dtypes
    xt = pre["xt"]
    sqnh = (pre["sqn"] / 2.0).astype(np.float32)
    sqn2 = _hilo(sqnh)
    in_maps = []
    for c in range(NCORES):
        nposx = np.empty((D, NRB * POSW), dtype=np.float32)
        npossqn = np.empty((NRB * POSW,), dtype=np.float32)
        for m in range(NRB):
            w0 = pre["w0"][NRB * c + m]
            nposx[:, POSW * m : POSW * (m + 1)] = -xt[:, w0 : w0 + POSW]
            npossqn[POSW * m : POSW * (m + 1)] = -sqnh[w0 : w0 + POSW]
        in_maps.append({
            "xt": xt,
            "sqn2": sqn2,
            "rowx": np.ascontiguousarray(xt[:, RPC * c : RPC * (c + 1)]),
            "negones": np.full((2, 128), -1.0, dtype=ml_dtypes.bfloat16),
            "nposx": nposx,
            "npossqn": _hilo(npossqn),
            "ns": np.concatenate(
                [_core_cols(v, c) for v in pre["negms"]], axis=1),
            "ne": np.concatenate(
                [_core_cols(v, c) for v in pre["negme"]], axis=1),
            "ps": _core_cols(pre["pstart"], c),
            "pe": _core_cols(pre["pend"], c),
        })
    return in_maps


def kernel(embeddings, labels):
    from concourse.bass_utils import run_bass_kernel_spmd

    pre = _prepare(embeddings, labels)
    if "nc" not in _cached:
        _cached["nc"] = _build_program()
    nc = _cached["nc"]

    res = run_bass_kernel_spmd(nc, _in_maps(pre), core_ids=list(range(NCORES)))

    accn = np.empty(B, dtype=np.float64)
    accp = np.empty(B, dtype=np.float64)
    for c in range(NCORES):
        a = res.results[c]["acc"]
        an = a[:, 0:NRB].copy()
        for g in range(1, NG):
            np.maximum(an, a[:, NRB * g : NRB * (g + 1)], out=an)
        accn[RPC * c : RPC * (c + 1)] = an.T.reshape(-1)
        accp[RPC * c : RPC * (c + 1)] = \
            a[:, NG * NRB : (NG + 1) * NRB].T.reshape(-1)

    sqn = pre["sqn"].astype(np.float64)
    # accn is max over negatives of (-H/2)  =>  min over negatives of H
    # is -2*accn;  accp is max over positives of (+H/2).
    hn2 = sqn - 2.0 * accn
    hp2 = sqn + 2.0 * accp
    hn = np.sqrt(np.maximum(hn2, 1e-12))
    hp = np.sqrt(np.maximum(hp2, 1e-12))
    per_row = np.maximum(hp - hn + MARGIN, 0.0) * pre["valid"]
    loss = per_row.sum() / max(pre["n_valid"], 1.0)
    return np.float32(loss)

